# revision 21
# baseline (speedup 1.0000x reference)
"""Trainium2 Bass kernel for ComiRec dynamic-routing (CapsNet-style) layer.

Problem: B=1024, S=200, E=128, C=128, n_caps=4, 3 routing rounds.

Sharding (8 cores): core i handles capsule n = i//2 and batch half h = i%2
(512 batch rows). Capsules are fully independent in the reference math, so
there is no cross-core communication. Each core processes its 512 batch rows
in 4 chunks of 128 (the partition width).

Per-core dataflow (per 128-row chunk):
  stage A : u[b,s,c] = (mask*behaviors)[b,s,:] @ W[n,s]   (PE matmuls, bf16,
            fp32 PSUM) evacuated to SBUF as bf16 [b, s, c]; in the same pass
            a second accumulating matmul computes z0_raw[b,c] = sum_s u.
  round 0 : caps0 = squash(z0_raw / n_valid)  (squash folded into one
            per-partition scale alpha' = alpha * invZ).
  rounds  : delta[b,s] = sum_c u*capsE (broadcast TT mult + pair-tree reduce)
            logits += delta; coup = exp(masked logits - max) (ACT, fused
            sum); caps = squash(sum_s coup*u / Z) via 200 per-s
            tensor_scalar mults + pair-tree reduce over s.
Masked (invalid) positions contribute zero because behaviors are pre-masked
on the host, and get -3e38 added before the softmax.

The kernel() entry takes FULL inputs (as produced by the problem's
setup_inputs) and returns the FULL [1024, 4, 128] fp32 output.
"""

import numpy as np
import ml_dtypes

BF16 = ml_dtypes.bfloat16

B, S, E, C, NCAPS = 1024, 200, 128, 128, 4
NCORES = 8
BH = B // 2          # batch rows per core (one half)
P = 128              # partition width / chunk size
NCHUNK = BH // P     # 4 chunks per core
SBLK = 8             # s-tiles per DMA block
NEG = -3.0e38

_COMPILED = {}
DEBUG_TAPS = False
NROUNDS = 3  # debug knob: 0 = stage A + z0 only (emits caps=caps0)
Z0MM = True  # debug knob: accumulate z0 via second interleaved matmul
GP_MUL = 0   # s rows of each big mult offloaded to gpsimd (0 = off; tested
GP_TREE = 0  # 32/20: gpsimd ran ~5x slower than spec + DVE port contention)


def _emit(ctx, tc, nc):
    import concourse.bass as bass
    from concourse import mybir

    bf = mybir.dt.bfloat16
    f32 = mybir.dt.float32

    bmt = nc.dram_tensor("bmt", [NCHUNK, E, S, P], bf, kind="ExternalInput").ap()
    w = nc.dram_tensor("w", [E, S, C], bf, kind="ExternalInput").ap()
    mneg = nc.dram_tensor("mneg", [NCHUNK, P, S], f32, kind="ExternalInput").ap()
    invn = nc.dram_tensor("invn", [NCHUNK, P, 1], f32, kind="ExternalInput").ap()
    out = nc.dram_tensor("caps_out", [NCHUNK, P, C], f32, kind="ExternalOutput").ap()
    taps = {}
    if DEBUG_TAPS:
        taps["u0"] = nc.dram_tensor("dbg_u0", [P, S, C], bf, kind="ExternalOutput").ap()
        taps["z0"] = nc.dram_tensor("dbg_z0", [P, C], f32, kind="ExternalOutput").ap()
        taps["caps0"] = nc.dram_tensor("dbg_caps0", [P, C], f32, kind="ExternalOutput").ap()
        taps["logits1"] = nc.dram_tensor("dbg_logits1", [P, S], f32, kind="ExternalOutput").ap()
        taps["coup1"] = nc.dram_tensor("dbg_coup1", [P, S], f32, kind="ExternalOutput").ap()
        taps["zraw1"] = nc.dram_tensor("dbg_zraw1", [P, C], f32, kind="ExternalOutput").ap()
        taps["caps1"] = nc.dram_tensor("dbg_caps1", [P, C], f32, kind="ExternalOutput").ap()
        taps["logits2"] = nc.dram_tensor("dbg_logits2", [P, S], f32, kind="ExternalOutput").ap()
        taps["zraw2"] = nc.dram_tensor("dbg_zraw2", [P, C], f32, kind="ExternalOutput").ap()
        taps["invz1"] = nc.dram_tensor("dbg_invz1", [P, 1], f32, kind="ExternalOutput").ap()

    wpool = ctx.enter_context(tc.tile_pool(name="wstream", bufs=3))
    bmtpool = ctx.enter_context(tc.tile_pool(name="bmt", bufs=3))
    upool = ctx.enter_context(tc.tile_pool(name="u", bufs=2))
    wtpool = ctx.enter_context(tc.tile_pool(name="wtmp", bufs=1))
    smalls = ctx.enter_context(tc.tile_pool(name="smalls", bufs=2))
    pupool = ctx.enter_context(tc.tile_pool(name="pu", bufs=3, space="PSUM"))
    pzpool = ctx.enter_context(tc.tile_pool(name="pz", bufs=2, space="PSUM"))

    # w/bmt are pre-transposed on the host so each partition line (e) reads
    # 2KB-contiguous runs per 8-s block (vs 256B strided gathers before).
    w_esc = w
    bmt_esb = bmt

    def tree_reduce_s(src, dst):
        # src: [P, S, C] bf16, reduced in place over the s axis (pair adds);
        # dst: [P, C] fp32 gets the final level. Odd widths keep the middle
        # element in place (half = w//2 pairs, next width = ceil(w/2)).
        # The first (largest) level is split with gpsimd taking a tail slice.
        wlen = S
        first = True
        while wlen > 2:
            half = wlen // 2
            nxt = (wlen + 1) // 2
            if first and GP_TREE:
                cut = half - GP_TREE
                nc.vector.tensor_add(
                    src[:, 0:cut, :], src[:, 0:cut, :], src[:, nxt:nxt + cut, :]
                )
                nc.gpsimd.tensor_add(
                    src[:, cut:half, :], src[:, cut:half, :],
                    src[:, nxt + cut:nxt + half, :],
                )
            else:
                nc.vector.tensor_add(
                    src[:, 0:half, :], src[:, 0:half, :], src[:, nxt:nxt + half, :]
                )
            first = False
            wlen = nxt
        nc.vector.tensor_add(dst, src[:, 0, :], src[:, 1, :])

    def tree_reduce_c(src, dst):
        # src: [P, S, C] bf16, reduced in place over the c axis; dst: [P, S] f32.
        # First level split: gpsimd handles a tail range of s rows.
        wlen = C
        first = True
        while wlen > 2:
            half = wlen // 2
            nxt = (wlen + 1) // 2
            if first and GP_TREE:
                cut = S - 2 * GP_TREE
                nc.vector.tensor_add(
                    src[:, 0:cut, 0:half], src[:, 0:cut, 0:half],
                    src[:, 0:cut, nxt:nxt + half],
                )
                nc.gpsimd.tensor_add(
                    src[:, cut:S, 0:half], src[:, cut:S, 0:half],
                    src[:, cut:S, nxt:nxt + half],
                )
            else:
                nc.vector.tensor_add(
                    src[:, :, 0:half], src[:, :, 0:half], src[:, :, nxt:nxt + half]
                )
            first = False
            wlen = nxt
        nc.vector.tensor_add(dst, src[:, :, 0], src[:, :, 1])

    def emit_sq(zraw):
        # n2raw = sum_c zraw^2 (ACT; emitted early so it overlaps DVE mults)
        sqt = smalls.tile([P, C], f32, tag="sqt")
        n2 = smalls.tile([P, 1], f32, tag="n2")
        nc.scalar.activation(
            out=sqt, in_=zraw, func=mybir.ActivationFunctionType.Square,
            accum_out=n2,
        )
        return n2

    def emit_alphap(n2, invz):
        # alpha' = alpha(n2') * invz with n2' = n2*invz^2; caps = zraw*alpha'
        n2t = smalls.tile([P, 1], f32, tag="n2t")
        nc.vector.tensor_scalar(
            out=n2t, in0=n2, scalar1=invz, scalar2=invz,
            op0=mybir.AluOpType.mult, op1=mybir.AluOpType.mult,
        )
        st = smalls.tile([P, 1], f32, tag="st")
        nc.scalar.sqrt(st, n2t)
        den = smalls.tile([P, 1], f32, tag="den")
        nc.vector.scalar_tensor_tensor(
            out=den, in0=n2t, scalar=1.0, in1=st,
            op0=mybir.AluOpType.add, op1=mybir.AluOpType.mult,
        )
        rd = smalls.tile([P, 1], f32, tag="rd")
        nc.vector.reciprocal(rd, den)
        t1 = smalls.tile([P, 1], f32, tag="t1")
        nc.vector.tensor_scalar_mul(t1, n2t, invz)
        alphap = smalls.tile([P, 1], f32, tag="alphap")
        nc.vector.tensor_mul(alphap, t1, rd)
        return alphap

    for k in range(NCHUNK):
        bsl = slice(k * P, (k + 1) * P)

        mneg_sb = smalls.tile([P, S], f32, tag="mneg")
        nc.sync.dma_start(out=mneg_sb, in_=mneg[k])
        invn_sb = smalls.tile([P, 1], f32, tag="invn")
        nc.sync.dma_start(out=invn_sb, in_=invn[k])

        u = upool.tile([P, S, C], bf, tag="u")
        wtmp = wtpool.tile([P, S, C], bf, tag="wtmp")
        if Z0MM:
            pz = pzpool.tile([P, C], f32, tag="pz")
        else:
            pz = None

        # ---------- stage A ----------
        pu = None
        for blk in range(S // SBLK):
            bt = bmtpool.tile([E, SBLK, P], bf, tag="bt")
            wt_blk = wpool.tile([E, SBLK, C], bf, tag="wt")
            sl = slice(blk * SBLK, (blk + 1) * SBLK)
            nc.sync.dma_start(out=bt, in_=bmt_esb[k][:, sl, :])
            nc.sync.dma_start(out=wt_blk, in_=w_esc[:, sl, :])
            for j in range(SBLK):
                s = blk * SBLK + j
                q = s % 8
                if q == 0:
                    pu = pupool.tile([P, 8 * C], f32, tag="pu")
                nc.tensor.matmul(
                    pu[:, q * C:(q + 1) * C], lhsT=bt[:, j, :], rhs=wt_blk[:, j, :],
                    start=True, stop=True,
                )
                if Z0MM:
                    nc.tensor.matmul(
                        pz, lhsT=bt[:, j, :], rhs=wt_blk[:, j, :],
                        start=(s == 0), stop=(s == S - 1), skip_group_check=True,
                    )
                if q == 7:
                    pv = pu.rearrange("p (a c) -> p a c", c=C)
                    dst = u[:, s - 7:s + 1, :]
                    nc.scalar.copy(dst, pv)

        logits = smalls.tile([P, S], f32, tag="logits")
        caps = smalls.tile([P, C], f32, tag="caps")
        capsb = smalls.tile([P, C], bf, tag="capsb")
        zraw = smalls.tile([P, C], f32, tag="zraw")
        invz = smalls.tile([P, 1], f32, tag="invz")

        if DEBUG_TAPS and k == 0:
            nc.sync.dma_start(out=taps["u0"], in_=u)

        for r in range(max(NROUNDS, 1)):
            if r == 0:
                if Z0MM:
                    nc.scalar.copy(zraw, pz)
                else:
                    nc.vector.memset(zraw, 1.0)
                invzv = invn_sb
                if DEBUG_TAPS and k == 0:
                    nc.sync.dma_start(out=taps["z0"], in_=zraw)
            else:
                # softmax (unnormalized, no max-subtraction: |logits| <= ~13
                # since squash keeps |caps| < 1 and |u_s| ~ 6.5, so exp stays
                # in fp32/bf16 range). exp is written twice, into adjacent
                # bf16 pair slots, so the caps_pre mult below can broadcast
                # along c with innermost stride 1 (keeps DVE 2x packing).
                zsum = smalls.tile([P, 1], f32, tag="zsum")
                coup2 = smalls.tile([P, S, 2], bf, tag="coup2")
                nc.scalar.activation(
                    out=coup2[:, :, 0], in_=logits,
                    func=mybir.ActivationFunctionType.Exp, accum_out=zsum,
                )
                nc.scalar.activation(
                    out=coup2[:, :, 1], in_=logits,
                    func=mybir.ActivationFunctionType.Exp,
                )
                nc.vector.reciprocal(invz, zsum)
                if DEBUG_TAPS and k == 0 and r == 1:
                    nc.sync.dma_start(out=taps["logits1"], in_=logits)
                u_pair = u.rearrange("p s (h t) -> p s h t", t=2)
                w_pair = wtmp.rearrange("p s (h t) -> p s h t", t=2)
                c2b = coup2.unsqueeze(2).broadcast_to([P, S, C // 2, 2])
                cut = S - GP_MUL
                nc.vector.tensor_mul(
                    w_pair[:, 0:cut], u_pair[:, 0:cut], c2b[:, 0:cut]
                )
                if GP_MUL:
                    nc.gpsimd.tensor_mul(
                        w_pair[:, cut:S], u_pair[:, cut:S], c2b[:, cut:S]
                    )
                tree_reduce_s(wtmp, zraw)
                invzv = invz
                if DEBUG_TAPS and k == 0 and r == 1:
                    nc.sync.dma_start(out=taps["zraw1"], in_=zraw)
                    nc.sync.dma_start(out=taps["invz1"], in_=invz)
                if DEBUG_TAPS and k == 0 and r == 2:
                    nc.sync.dma_start(out=taps["zraw2"], in_=zraw)

            n2 = emit_sq(zraw)
            if r < min(2, NROUNDS - 1):
                # delta mult uses raw z (squash scale alpha' is folded into the
                # logits update below), so the squash chain overlaps the mult.
                nc.vector.tensor_copy(capsb, zraw)
                capse = capsb.unsqueeze(1).broadcast_to([P, S, C])
                cutm = S - GP_MUL
                nc.vector.tensor_mul(
                    wtmp[:, 0:cutm, :], u[:, 0:cutm, :], capse[:, 0:cutm, :]
                )
                if GP_MUL:
                    nc.gpsimd.tensor_mul(
                        wtmp[:, cutm:S, :], u[:, cutm:S, :], capse[:, cutm:S, :]
                    )
                alphap = emit_alphap(n2, invzv)
                dtmp = smalls.tile([P, S], f32, tag="dtmp")
                tree_reduce_c(wtmp, dtmp)
                if r == 0:
                    # logits = alphap*dtmp + mneg: the -3e38 mask bias is
                    # folded in once here; later rounds add deltas on top and
                    # exp() reads logits directly (invalid stays ~-3e38 -> 0).
                    nc.vector.scalar_tensor_tensor(
                        out=logits, in0=dtmp, scalar=alphap, in1=mneg_sb,
                        op0=mybir.AluOpType.mult, op1=mybir.AluOpType.add,
                    )
                else:
                    nc.vector.scalar_tensor_tensor(
                        out=logits, in0=dtmp, scalar=alphap, in1=logits,
                        op0=mybir.AluOpType.mult, op1=mybir.AluOpType.add,
                    )
                    if DEBUG_TAPS and k == 0:
                        nc.sync.dma_start(out=taps["logits2"], in_=logits)
            else:
                alphap = emit_alphap(n2, invzv)
                nc.vector.tensor_scalar_mul(caps, zraw, alphap)

        outsb = smalls.tile([P, C], f32, tag="outsb")
        nc.vector.tensor_copy(outsb, caps)
        nc.sync.dma_start(out=out[k], in_=outsb)


def _build():
    if "nc" in _COMPILED:
        return _COMPILED["nc"]
    from contextlib import ExitStack
    import concourse.bacc as bacc
    import concourse.tile as tile

    nc = bacc.Bacc(
        "TRN2", target_bir_lowering=False, debug=False, enable_asserts=False
    )
    with tile.TileContext(nc, trace_sim=False) as tc, ExitStack() as ctx:
        _emit(ctx, tc, nc)
    nc.compile()
    _COMPILED["nc"] = nc
    return nc


def make_in_maps(behaviors, valid_mask, W):
    behaviors = np.asarray(behaviors, dtype=np.float32)
    mask = np.asarray(valid_mask).astype(bool)
    W = np.asarray(W, dtype=np.float32)

    bm = (behaviors * mask[:, :, None].astype(np.float32)).astype(BF16)  # [B,S,E]
    w_bf = np.ascontiguousarray(W.transpose(0, 2, 1, 3)).astype(BF16)    # [N,E,S,C]
    mneg_full = np.where(mask, 0.0, NEG).astype(np.float32)              # [B,S]
    nval = mask.sum(axis=1).astype(np.float32)
    invn_full = (1.0 / np.maximum(nval, 1.0)).astype(np.float32)         # [B]

    in_maps = []
    for core in range(NCORES):
        n, h = core // 2, core % 2
        bsl = slice(h * BH, (h + 1) * BH)
        bmt = bm[bsl].reshape(NCHUNK, P, S, E).transpose(0, 3, 2, 1)
        in_maps.append({
            "bmt": np.ascontiguousarray(bmt),                # [NCHUNK,E,S,P]
            "w": w_bf[n],
            "mneg": np.ascontiguousarray(
                mneg_full[bsl].reshape(NCHUNK, P, S)),
            "invn": np.ascontiguousarray(
                invn_full[bsl].reshape(NCHUNK, P, 1)),
        })
    return in_maps


def gather_output(results):
    out = np.empty((B, NCAPS, C), dtype=np.float32)
    for core in range(NCORES):
        n, h = core // 2, core % 2
        caps = results[core]["caps_out"].reshape(BH, C)
        out[h * BH:(h + 1) * BH, n, :] = caps
    return out


def kernel(behaviors, valid_mask, W):
    from concourse import bass_utils

    nc = _build()
    in_maps = make_in_maps(behaviors, valid_mask, W)
    res = bass_utils.run_bass_kernel_spmd(nc, in_maps, core_ids=list(range(NCORES)))
    return gather_output(res.results)



# revision 25
# speedup vs baseline: 1.0589x; 1.0589x over previous
"""Trainium2 Bass kernel for ComiRec dynamic-routing (CapsNet-style) layer.

Problem: B=1024, S=200, E=128, C=128, n_caps=4, 3 routing rounds.

Sharding (8 cores): core i handles capsule n = i//2 and batch half h = i%2
(512 batch rows). Capsules are fully independent in the reference math, so
there is no cross-core communication. Each core processes its 512 batch rows
in 4 chunks of 128 (the partition width).

Per-core dataflow (per 128-row chunk):
  stage A : u[b,s,c] = (mask*behaviors)[b,s,:] @ W[n,s]   (PE matmuls, bf16,
            fp32 PSUM) evacuated to SBUF as bf16 [b, s, c]; in the same pass
            a second accumulating matmul computes z0_raw[b,c] = sum_s u.
  round 0 : caps0 = squash(z0_raw / n_valid)  (squash folded into one
            per-partition scale alpha' = alpha * invZ).
  rounds  : delta[b,s] = sum_c u*capsE (broadcast TT mult + pair-tree reduce)
            logits += delta; coup = exp(masked logits - max) (ACT, fused
            sum); caps = squash(sum_s coup*u / Z) via 200 per-s
            tensor_scalar mults + pair-tree reduce over s.
Masked (invalid) positions contribute zero because behaviors are pre-masked
on the host, and get -3e38 added before the softmax.

The kernel() entry takes FULL inputs (as produced by the problem's
setup_inputs) and returns the FULL [1024, 4, 128] fp32 output.
"""

import numpy as np
import ml_dtypes

BF16 = ml_dtypes.bfloat16

B, S, E, C, NCAPS = 1024, 200, 128, 128, 4
NCORES = 8
BH = B // 2          # batch rows per core (one half)
P = 128              # partition width / chunk size
NCHUNK = BH // P     # 4 chunks per core
SBLK = 8             # s-tiles per DMA block
NEG = -3.0e38

_COMPILED = {}
DEBUG_TAPS = False
NROUNDS = 3  # debug knob: 0 = stage A + z0 only (emits caps=caps0)
Z0MM = True  # debug knob: accumulate z0 via second interleaved matmul
GP_MUL = 0   # s rows of each big mult offloaded to gpsimd (0 = off; tested
GP_TREE = 0  # 32/20: gpsimd ran ~5x slower than spec + DVE port contention)


def _emit(ctx, tc, nc):
    import concourse.bass as bass
    from concourse import mybir

    bf = mybir.dt.bfloat16
    f32 = mybir.dt.float32

    bmt = nc.dram_tensor("bmt", [S, E, BH], bf, kind="ExternalInput").ap()
    w = nc.dram_tensor("w", [S, E, C], bf, kind="ExternalInput").ap()
    mneg = nc.dram_tensor("mneg", [NCHUNK, P, S], f32, kind="ExternalInput").ap()
    invn = nc.dram_tensor("invn", [NCHUNK, P, 1], f32, kind="ExternalInput").ap()
    out = nc.dram_tensor("caps_out", [NCHUNK, P, C], f32, kind="ExternalOutput").ap()
    taps = {}
    if DEBUG_TAPS:
        taps["u0"] = nc.dram_tensor("dbg_u0", [P, S, C], bf, kind="ExternalOutput").ap()
        taps["z0"] = nc.dram_tensor("dbg_z0", [P, C], f32, kind="ExternalOutput").ap()
        taps["caps0"] = nc.dram_tensor("dbg_caps0", [P, C], f32, kind="ExternalOutput").ap()
        taps["logits1"] = nc.dram_tensor("dbg_logits1", [P, S], f32, kind="ExternalOutput").ap()
        taps["coup1"] = nc.dram_tensor("dbg_coup1", [P, S], f32, kind="ExternalOutput").ap()
        taps["zraw1"] = nc.dram_tensor("dbg_zraw1", [P, C], f32, kind="ExternalOutput").ap()
        taps["caps1"] = nc.dram_tensor("dbg_caps1", [P, C], f32, kind="ExternalOutput").ap()
        taps["logits2"] = nc.dram_tensor("dbg_logits2", [P, S], f32, kind="ExternalOutput").ap()
        taps["zraw2"] = nc.dram_tensor("dbg_zraw2", [P, C], f32, kind="ExternalOutput").ap()
        taps["invz1"] = nc.dram_tensor("dbg_invz1", [P, 1], f32, kind="ExternalOutput").ap()

    wpool = ctx.enter_context(tc.tile_pool(name="wstream", bufs=3))
    bmtpool = ctx.enter_context(tc.tile_pool(name="bmt", bufs=3))
    upool = ctx.enter_context(tc.tile_pool(name="u", bufs=2))
    wtpool = ctx.enter_context(tc.tile_pool(name="wtmp", bufs=1))
    smalls = ctx.enter_context(tc.tile_pool(name="smalls", bufs=2))
    pupool = ctx.enter_context(tc.tile_pool(name="pu", bufs=3, space="PSUM"))
    pzpool = ctx.enter_context(tc.tile_pool(name="pz", bufs=2, space="PSUM"))

    w_esc = w.rearrange("s e c -> e s c")
    bmt_esb = bmt.rearrange("s e b -> e s b")

    def tree_reduce_s(src, dst):
        # src: [P, S, C] bf16, reduced in place over the s axis (pair adds);
        # dst: [P, C] fp32 gets the final level. Odd widths keep the middle
        # element in place (half = w//2 pairs, next width = ceil(w/2)).
        # The first (largest) level is split with gpsimd taking a tail slice.
        wlen = S
        first = True
        while wlen > 2:
            half = wlen // 2
            nxt = (wlen + 1) // 2
            if first and GP_TREE:
                cut = half - GP_TREE
                nc.vector.tensor_add(
                    src[:, 0:cut, :], src[:, 0:cut, :], src[:, nxt:nxt + cut, :]
                )
                nc.gpsimd.tensor_add(
                    src[:, cut:half, :], src[:, cut:half, :],
                    src[:, nxt + cut:nxt + half, :],
                )
            else:
                nc.vector.tensor_add(
                    src[:, 0:half, :], src[:, 0:half, :], src[:, nxt:nxt + half, :]
                )
            first = False
            wlen = nxt
        nc.vector.tensor_add(dst, src[:, 0, :], src[:, 1, :])

    def tree_reduce_c(src, dst):
        # src: [P, S, C] bf16, reduced in place over the c axis; dst: [P, S] f32.
        # First level split: gpsimd handles a tail range of s rows.
        wlen = C
        first = True
        while wlen > 2:
            half = wlen // 2
            nxt = (wlen + 1) // 2
            if first and GP_TREE:
                cut = S - 2 * GP_TREE
                nc.vector.tensor_add(
                    src[:, 0:cut, 0:half], src[:, 0:cut, 0:half],
                    src[:, 0:cut, nxt:nxt + half],
                )
                nc.gpsimd.tensor_add(
                    src[:, cut:S, 0:half], src[:, cut:S, 0:half],
                    src[:, cut:S, nxt:nxt + half],
                )
            else:
                nc.vector.tensor_add(
                    src[:, :, 0:half], src[:, :, 0:half], src[:, :, nxt:nxt + half]
                )
            first = False
            wlen = nxt
        nc.vector.tensor_add(dst, src[:, :, 0], src[:, :, 1])

    def emit_sq(zraw):
        # n2raw = sum_c zraw^2 (ACT; emitted early so it overlaps DVE mults)
        sqt = smalls.tile([P, C], f32, tag="sqt")
        n2 = smalls.tile([P, 1], f32, tag="n2")
        nc.scalar.activation(
            out=sqt, in_=zraw, func=mybir.ActivationFunctionType.Square,
            accum_out=n2,
        )
        return n2

    def emit_alphap(n2, invz):
        # alpha' = alpha(n2') * invz with n2' = n2*invz^2; caps = zraw*alpha'
        n2t = smalls.tile([P, 1], f32, tag="n2t")
        nc.vector.tensor_scalar(
            out=n2t, in0=n2, scalar1=invz, scalar2=invz,
            op0=mybir.AluOpType.mult, op1=mybir.AluOpType.mult,
        )
        st = smalls.tile([P, 1], f32, tag="st")
        nc.scalar.sqrt(st, n2t)
        den = smalls.tile([P, 1], f32, tag="den")
        nc.vector.scalar_tensor_tensor(
            out=den, in0=n2t, scalar=1.0, in1=st,
            op0=mybir.AluOpType.add, op1=mybir.AluOpType.mult,
        )
        rd = smalls.tile([P, 1], f32, tag="rd")
        nc.vector.reciprocal(rd, den)
        t1 = smalls.tile([P, 1], f32, tag="t1")
        nc.vector.tensor_scalar_mul(t1, n2t, invz)
        alphap = smalls.tile([P, 1], f32, tag="alphap")
        nc.vector.tensor_mul(alphap, t1, rd)
        return alphap

    for k in range(NCHUNK):
        bsl = slice(k * P, (k + 1) * P)

        mneg_sb = smalls.tile([P, S], f32, tag="mneg")
        nc.sync.dma_start(out=mneg_sb, in_=mneg[k])
        invn_sb = smalls.tile([P, 1], f32, tag="invn")
        nc.sync.dma_start(out=invn_sb, in_=invn[k])

        u = upool.tile([P, S, C], bf, tag="u")
        wtmp = wtpool.tile([P, S, C], bf, tag="wtmp")
        if Z0MM:
            pz = pzpool.tile([P, C], f32, tag="pz")
        else:
            pz = None

        # ---------- stage A ----------
        pu = None
        for blk in range(S // SBLK):
            bt = bmtpool.tile([E, SBLK, P], bf, tag="bt")
            wt_blk = wpool.tile([E, SBLK, C], bf, tag="wt")
            sl = slice(blk * SBLK, (blk + 1) * SBLK)
            nc.sync.dma_start(out=bt, in_=bmt_esb[:, sl, bsl])
            nc.sync.dma_start(out=wt_blk, in_=w_esc[:, sl, :])
            for j in range(SBLK):
                s = blk * SBLK + j
                q = s % 8
                if q == 0:
                    pu = pupool.tile([P, 8 * C], f32, tag="pu")
                nc.tensor.matmul(
                    pu[:, q * C:(q + 1) * C], lhsT=bt[:, j, :], rhs=wt_blk[:, j, :],
                    start=True, stop=True,
                )
                if Z0MM:
                    nc.tensor.matmul(
                        pz, lhsT=bt[:, j, :], rhs=wt_blk[:, j, :],
                        start=(s == 0), stop=(s == S - 1), skip_group_check=True,
                    )
                if q == 7:
                    pv = pu.rearrange("p (a c) -> p a c", c=C)
                    dst = u[:, s - 7:s + 1, :]
                    nc.scalar.copy(dst, pv)

        logits = smalls.tile([P, S], f32, tag="logits")
        caps = smalls.tile([P, C], f32, tag="caps")
        capsb = smalls.tile([P, C], bf, tag="capsb")
        zraw = smalls.tile([P, C], f32, tag="zraw")
        invz = smalls.tile([P, 1], f32, tag="invz")

        if DEBUG_TAPS and k == 0:
            nc.sync.dma_start(out=taps["u0"], in_=u)

        for r in range(max(NROUNDS, 1)):
            if r == 0:
                if Z0MM:
                    nc.scalar.copy(zraw, pz)
                else:
                    nc.vector.memset(zraw, 1.0)
                invzv = invn_sb
                if DEBUG_TAPS and k == 0:
                    nc.sync.dma_start(out=taps["z0"], in_=zraw)
            else:
                # softmax (unnormalized, no max-subtraction: |logits| <= ~13
                # since squash keeps |caps| < 1 and |u_s| ~ 6.5, so exp stays
                # in fp32/bf16 range). exp is written twice, into adjacent
                # bf16 pair slots, so the caps_pre mult below can broadcast
                # along c with innermost stride 1 (keeps DVE 2x packing).
                zsum = smalls.tile([P, 1], f32, tag="zsum")
                coup2 = smalls.tile([P, S, 2], bf, tag="coup2")
                nc.scalar.activation(
                    out=coup2[:, :, 0], in_=logits,
                    func=mybir.ActivationFunctionType.Exp, accum_out=zsum,
                )
                nc.scalar.activation(
                    out=coup2[:, :, 1], in_=logits,
                    func=mybir.ActivationFunctionType.Exp,
                )
                nc.vector.reciprocal(invz, zsum)
                if DEBUG_TAPS and k == 0 and r == 1:
                    nc.sync.dma_start(out=taps["logits1"], in_=logits)
                u_pair = u.rearrange("p s (h t) -> p s h t", t=2)
                w_pair = wtmp.rearrange("p s (h t) -> p s h t", t=2)
                c2b = coup2.unsqueeze(2).broadcast_to([P, S, C // 2, 2])
                cut = S - GP_MUL
                nc.vector.tensor_mul(
                    w_pair[:, 0:cut], u_pair[:, 0:cut], c2b[:, 0:cut]
                )
                if GP_MUL:
                    nc.gpsimd.tensor_mul(
                        w_pair[:, cut:S], u_pair[:, cut:S], c2b[:, cut:S]
                    )
                tree_reduce_s(wtmp, zraw)
                invzv = invz
                if DEBUG_TAPS and k == 0 and r == 1:
                    nc.sync.dma_start(out=taps["zraw1"], in_=zraw)
                    nc.sync.dma_start(out=taps["invz1"], in_=invz)
                if DEBUG_TAPS and k == 0 and r == 2:
                    nc.sync.dma_start(out=taps["zraw2"], in_=zraw)

            n2 = emit_sq(zraw)
            if r < min(2, NROUNDS - 1):
                # delta mult uses raw z (squash scale alpha' is folded into the
                # logits update below), so the squash chain overlaps the mult.
                nc.vector.tensor_copy(capsb, zraw)
                capse = capsb.unsqueeze(1).broadcast_to([P, S, C])
                cutm = S - GP_MUL
                nc.vector.tensor_mul(
                    wtmp[:, 0:cutm, :], u[:, 0:cutm, :], capse[:, 0:cutm, :]
                )
                if GP_MUL:
                    nc.gpsimd.tensor_mul(
                        wtmp[:, cutm:S, :], u[:, cutm:S, :], capse[:, cutm:S, :]
                    )
                alphap = emit_alphap(n2, invzv)
                dtmp = smalls.tile([P, S], f32, tag="dtmp")
                tree_reduce_c(wtmp, dtmp)
                if r == 0:
                    # logits = alphap*dtmp + mneg: the -3e38 mask bias is
                    # folded in once here; later rounds add deltas on top and
                    # exp() reads logits directly (invalid stays ~-3e38 -> 0).
                    nc.vector.scalar_tensor_tensor(
                        out=logits, in0=dtmp, scalar=alphap, in1=mneg_sb,
                        op0=mybir.AluOpType.mult, op1=mybir.AluOpType.add,
                    )
                else:
                    nc.vector.scalar_tensor_tensor(
                        out=logits, in0=dtmp, scalar=alphap, in1=logits,
                        op0=mybir.AluOpType.mult, op1=mybir.AluOpType.add,
                    )
                    if DEBUG_TAPS and k == 0:
                        nc.sync.dma_start(out=taps["logits2"], in_=logits)
            else:
                alphap = emit_alphap(n2, invzv)
                nc.vector.tensor_scalar_mul(caps, zraw, alphap)

        outsb = smalls.tile([P, C], f32, tag="outsb")
        nc.vector.tensor_copy(outsb, caps)
        nc.sync.dma_start(out=out[k], in_=outsb)


def _build():
    if "nc" in _COMPILED:
        return _COMPILED["nc"]
    from contextlib import ExitStack
    import concourse.bacc as bacc
    import concourse.tile as tile

    nc = bacc.Bacc(
        "TRN2", target_bir_lowering=False, debug=False, enable_asserts=False
    )
    with tile.TileContext(nc, trace_sim=False) as tc, ExitStack() as ctx:
        _emit(ctx, tc, nc)
    nc.compile()
    _COMPILED["nc"] = nc
    return nc


def make_in_maps(behaviors, valid_mask, W):
    behaviors = np.asarray(behaviors, dtype=np.float32)
    mask = np.asarray(valid_mask).astype(bool)
    W = np.asarray(W, dtype=np.float32)

    bm = behaviors * mask[:, :, None].astype(np.float32)
    bmt_full = np.ascontiguousarray(bm.transpose(1, 2, 0)).astype(BF16)  # [S,E,B]
    w_bf = W.astype(BF16)                                               # [N,S,E,C]
    mneg_full = np.where(mask, 0.0, NEG).astype(np.float32)             # [B,S]
    nval = mask.sum(axis=1).astype(np.float32)
    invn_full = (1.0 / np.maximum(nval, 1.0)).astype(np.float32)        # [B]

    in_maps = []
    for core in range(NCORES):
        n, h = core // 2, core % 2
        bsl = slice(h * BH, (h + 1) * BH)
        in_maps.append({
            "bmt": np.ascontiguousarray(bmt_full[:, :, bsl]),
            "w": np.ascontiguousarray(w_bf[n]),
            "mneg": np.ascontiguousarray(
                mneg_full[bsl].reshape(NCHUNK, P, S)),
            "invn": np.ascontiguousarray(
                invn_full[bsl].reshape(NCHUNK, P, 1)),
        })
    return in_maps


def gather_output(results):
    out = np.empty((B, NCAPS, C), dtype=np.float32)
    for core in range(NCORES):
        n, h = core // 2, core % 2
        caps = results[core]["caps_out"].reshape(BH, C)
        out[h * BH:(h + 1) * BH, n, :] = caps
    return out


def kernel(behaviors, valid_mask, W):
    from concourse import bass_utils

    nc = _build()
    in_maps = make_in_maps(behaviors, valid_mask, W)
    res = bass_utils.run_bass_kernel_spmd(nc, in_maps, core_ids=list(range(NCORES)))
    return gather_output(res.results)

